# revision 10
# baseline (speedup 1.0000x reference)
"""Trainium2 Bass kernel for batched attention with softmax over the query axis.

Reference computation (per batch element b):
    Q = tokens @ Wq; K = tokens @ Wk; V = tokens @ Wv
    S = Q @ K.T                [T(t), T(s)]
    A = softmax(S, axis=t)     (normalizes over the *query* axis per key column)
    out = A @ V                [T, H]

Sharding: pure data parallelism - B=8 batch elements, one per NeuronCore.
The softmax couples queries only within a batch element, so no collectives.

Per-core implementation (fp16 matmul operands, fp32 PSUM accumulation):
  - W_qk = Wq @ Wk.T is built on-chip (weight-only work that overlaps the
    token DMA), so scores need one projection G = tokens @ W_qk instead of
    separate Q and K: S = G @ tokens.T.
  - All input DMAs flow through one ring of [128,512] f32 chunk tiles on the
    Sync queue in dependency-priority order (wq, wk, tok stage 0, wv, tok
    stages 1-3); slot reuse orders later transfers behind earlier consumers.
  - Weights are transposed on the PE in f32 straight off the DMA stage (PE is
    DMA-paced idle there; the batched DVE evacuation doubles as the f16 cast,
    keeping the DVE off the W_qk critical path). Tokens are DVE-cast to f16
    then PE-transposed (tokT) with one batched DVE evac per t-tile.
  - GT   [g%128, g//128, t] via lhsT=W_qk rhs=tokT.
  - V    [s%128, s//128, h] via lhsT=tokT rhs=Wv; evacuated by ScalarE so the
    next stage's t16 casts stay at the DVE queue head (no stage-boundary PE
    bubble). Stage 0 runs GT before V (wv arrives later); stages 1-3 run V
    first (V tile st needs only the first tokT evacuation of the stage).
  - S_st [s%128, t] via lhsT=tokT rhs=GT -> softmax over t is a free-axis
    reduction: max (DVE), exp (ScalarE, accum_out produces row sums).
  - 1/rowsum is folded into V rows, so the unnormalized exp tile E feeds the
    context matmul: ctx[t,h] via lhsT=E rhs=V' accumulated over s.
The matmul stream runs gapless at the 216ns/512-free cadence from the end of
prep to the last context matmul; engine assignment is balanced so DVE/ScalarE
softmax work sustains that cadence.

NOTE: several seemingly-better variants deterministically flip the chip to a
20% slower clock for the whole NEFF (extra ScalarE casts in prep, fp8
DoubleRow matmuls, DMA-crossbar transposes). Compare runs by matmul cadence
(216ns fast vs 259ns slow), not wall time alone, before believing a result.
"""

import numpy as np

import concourse.bass as bass
import concourse.bacc as bacc
import concourse.tile as tile
from concourse import mybir
from concourse.bass_utils import run_bass_kernel_spmd
from concourse.masks import make_identity

B, T, H, E = 8, 2048, 512, 512
P = 128
NT = T // P      # 16 tiles along t / s
NH = H // P      # 4 tiles along h
FD = 512         # matmul moving free dim (one fp32 PSUM bank)
NC_T = T // FD   # 4 free-dim chunks along t
NST = T // FD    # 4 token stage groups (4 t-tiles each)

F32 = mybir.dt.float32
F16 = mybir.dt.float16
AX = mybir.AxisListType
AF = mybir.ActivationFunctionType

N_CORES = 8


def build():
    nc = bacc.Bacc()
    tok_d = nc.declare_dram_parameter("tokens", [T, H], F32, isOutput=False)
    wq_d = nc.declare_dram_parameter("Wq", [H, E], F32, isOutput=False)
    wk_d = nc.declare_dram_parameter("Wk", [H, E], F32, isOutput=False)
    wv_d = nc.declare_dram_parameter("Wv", [H, H], F32, isOutput=False)
    out_d = nc.declare_dram_parameter("out", [T, H], F32, isOutput=True)

    # [p, tt, h]: partition = t%128, stage groups of 4 t-tiles -> 1MB DMAs
    tok_staged = tok_d.rearrange("(sg tt p) h -> sg p tt h", p=P, tt=NT // NST)
    out_tiled = out_d.rearrange("(tt p) h -> tt p h", p=P)

    with tile.TileContext(nc) as tc:
        with (
            tc.tile_pool(name="persist", bufs=1) as pp,
            tc.tile_pool(name="stage", bufs=2) as sp,
            tc.tile_pool(name="w16", bufs=4) as wp,
            tc.tile_pool(name="t16", bufs=6) as tp,
            tc.tile_pool(name="ostage", bufs=3) as osp,
            tc.tile_pool(name="stats", bufs=4) as stp,
            tc.tile_pool(name="psum", bufs=8, space=bass.MemorySpace.PSUM) as psp,
        ):
            ident = pp.tile([P, P], F16, tag="ident")
            make_identity(nc, ident[:])
            ident32 = pp.tile([P, P], F32, tag="ident32")
            make_identity(nc, ident32[:])

            # ---- HAM warm-up: the PE clock gate defaults to K=4/8
            # (1.2 GHz) and only reaches 8/8 after ~3.4us of sustained
            # REAL matmul activity (transpose-mode does not count).
            # Without this the whole prep phase runs at half clock
            # (trace: HAM K=8 only at ts~19us). Dummy ident x ident
            # matmuls bridge the DMA-bound head: a small front batch
            # (ident ready ~7.8us -> first wk chunk ~9.0us), plus 4
            # dummies after each wk-chunk transpose group to keep the
            # HAM activity window fed until Wqk's dense matmuls start.
            # Counts are sized to never head-of-line-block real work.
            ps_warm = psp.tile([P, P], F32, tag="mm", name="ps_warm")

            def warmup(n):
                for _ in range(n):
                    nc.tensor.matmul(ps_warm[:], ident[:], ident[:],
                                     start=True, stop=True)

            warmup(12)

            # ---- input DMA pushes (Sync queue) through ONE ring of 8
            # [128,512] f32 chunk tiles. Slot reuse makes each later DMA wait
            # until the earlier chunk in that slot has been consumed, so the
            # weight chunks get near-exclusive HBM bandwidth up front instead
            # of round-robin sharing with 4MB of token traffic.
            def chunk(name):
                return sp.tile([P, FD], F32, tag="st32", bufs=8, name=name)

            wstages = {"wq": [], "wk": []}
            wtiled = {
                "wq": wq_d.rearrange("(hh p) e -> hh p e", p=P),
                "wk": wk_d.rearrange("(hh p) e -> hh p e", p=P),
            }
            for name in ("wk", "wq"):
                for hh in range(NH):
                    ws = chunk(f"wst_{name}{hh}")
                    nc.sync.dma_start(ws[:], wtiled[name][hh])
                    wstages[name].append(ws)

            tok_chunked = tok_d.rearrange("(sg tt p) h -> sg tt p h", p=P,
                                          tt=NT // NST)
            tchunks = [[None] * (NT // NST) for _ in range(NST)]
            for ti in range(NT // NST):
                tc0 = chunk(f"tok0_{ti}")
                nc.sync.dma_start(tc0[:], tok_chunked[0][ti])
                tchunks[0][ti] = tc0

            wv_tiled = wv_d.rearrange("(hh p) e -> hh p e", p=P)
            wvchunks = []
            for hh in range(NH):
                wc = chunk(f"wv{hh}")
                nc.sync.dma_start(wc[:], wv_tiled[hh])
                wvchunks.append(wc)

            for sg in range(1, NST):
                for ti in range(NT // NST):
                    tcx = chunk(f"tok{sg}_{ti}")
                    nc.sync.dma_start(tcx[:], tok_chunked[sg][ti])
                    tchunks[sg][ti] = tcx

            # ---- Wq/Wk: f32 PE transpose straight off the DMA stage (PE
            # is DMA-paced here; the batched DVE evacuation doubles as
            # the f16 cast). Each W_qk[hb] matmul group is emitted right
            # after its wq-chunk transpose so dense real matmuls start as
            # soon as wq chunks land; wk groups get dummy-matmul filler
            # to keep the HAM window busy during DMA waits.
            wT16 = {
                "wq": pp.tile([P, NH, E], F16, tag="wqT", name="wT_wq"),
                "wk": pp.tile([P, NH, E], F16, tag="wkT", name="wT_wk"),
            }
            Wqk = pp.tile([P, NH, H], F16, tag="Wqk")
            for name in ("wk", "wq"):
                for hh in range(NH):
                    # f32 transpose straight off the stage: PE is DMA-paced
                    # idle there, and it keeps the DVE off the W_qk
                    # critical path (one batched evac per chunk is all).
                    ps_tr = psp.tile([P, NH, P], F32, tag="mm",
                                     name=f"tr_{name}{hh}")
                    for eb in range(NH):
                        nc.tensor.transpose(
                            ps_tr[:, eb],
                            wstages[name][hh][:, eb * P : (eb + 1) * P],
                            ident32[:],
                        )
                    nc.vector.tensor_copy(
                        wT16[name][:, :, hh * P : (hh + 1) * P], ps_tr[:]
                    )
                    # fill the DMA wait for the next chunk with real
                    # (HAM-visible) matmul activity; transposes alone do
                    # not count as PE-busy for the HAM clock gate
                    warmup(4 if name == "wk" else 2)

            # ---- W_qk = Wq @ Wk.T : [h%128, hb, h'] fp16 ----
            # Emitted batched after all transposes: interleaving it per
            # wq chunk was measured slower (the PE then waits out the
            # PSUM->SBUF evac latency once per chunk instead of once).
            for hb in range(NH):
                ps = psp.tile([P, FD], F32, tag="mm", name=f"ps_wqk{hb}")
                for eb in range(NH):
                    nc.tensor.matmul(
                        ps[:],
                        wT16["wq"][:, eb, hb * P : (hb + 1) * P],
                        wT16["wk"][:, eb, :],
                        start=(eb == 0),
                        stop=(eb == NH - 1),
                    )
                nc.scalar.copy(Wqk[:, hb, :], ps[:])

            # Token stage 0 casts on GpSimd (idle engine; keeps the DVE
            # free for the PSUM evacuation chain that gates GT0).
            t16s0 = []
            for ti in range(NT // NST):
                t16 = tp.tile([P, H], F16, tag="t16", name=f"t16_{ti}")
                nc.gpsimd.tensor_copy(t16[:], tchunks[0][ti][:])
                t16s0.append(t16)

            wv16 = pp.tile([P, NH, E], F16, tag="wv16")

            # ---- per stage: transposes -> GT chunk -> V tiles ----
            tokT = pp.tile([P, NH, T], F16, tag="tokT")
            GT = pp.tile([P, NH, T], F16, tag="GT")
            V = pp.tile([P, NT, H], F16, tag="V")
            for sg in range(NST):
                # t16 casts for this stage (stage 0 pre-cast above).
                if sg == 0:
                    t16s = t16s0
                else:
                    t16s = []
                    for ti in range(NT // NST):
                        t16 = tp.tile([P, H], F16, tag="t16",
                                      name=f"t16_{sg * (NT // NST) + ti}")
                        nc.gpsimd.tensor_copy(t16[:], tchunks[sg][ti][:])
                        t16s.append(t16)
                # Transposes for two t-tiles share one PSUM tile so the
                # stage needs 2 DVE evacuations instead of 4 (the evac
                # chain gates GT[sg]'s rhs).
                for pi in range(NT // NST // 2):
                    ps_tr = psp.tile([P, NH, 2 * P], F16, tag="mm",
                                     name=f"trt{sg}_{pi}")
                    for k in range(2):
                        ti = pi * 2 + k
                        for ht in range(NH):
                            nc.tensor.transpose(
                                ps_tr[:, ht, k * P : (k + 1) * P],
                                t16s[ti][:, ht * P : (ht + 1) * P],
                                ident[:],
                            )
                    base = sg * FD + pi * 2 * P
                    nc.vector.tensor_copy(
                        tokT[:, :, base : base + 2 * P], ps_tr[:]
                    )
                if sg == 0:
                    # wv casts on GpSimd, queued after the stage-0 token
                    # casts; ready before V[0] needs them.
                    for hh in range(NH):
                        nc.gpsimd.tensor_copy(wv16[:, hh], wvchunks[hh][:])

                def emit_V(sg):
                    for st in range(sg * NC_T, (sg + 1) * NC_T):
                        ps = psp.tile([P, FD], F32, tag="mm", name=f"ps_v{st}")
                        for ht in range(NH):
                            nc.tensor.matmul(
                                ps[:],
                                tokT[:, ht, st * P : (st + 1) * P],
                                wv16[:, ht, :],
                                start=(ht == 0),
                                stop=(ht == NH - 1),
                            )
                        nc.scalar.copy(V[:, st, :], ps[:])

                def emit_GT(sg):
                    # hb-outer: the first 12 matmuls need only Wqk[0..2],
                    # so GT[0] isn't stalled on Wqk[3]'s ScalarE evac.
                    tch = sg
                    pss = [
                        psp.tile([P, FD], F32, tag="mm",
                                 name=f"ps_g{gb}_{tch}")
                        for gb in range(NH)
                    ]
                    for hb in range(NH):
                        for gb in range(NH):
                            nc.tensor.matmul(
                                pss[gb][:],
                                Wqk[:, hb, gb * P : (gb + 1) * P],
                                tokT[:, hb, tch * FD : (tch + 1) * FD],
                                start=(hb == 0),
                                stop=(hb == NH - 1),
                            )
                    for gb in range(NH):
                        nc.scalar.copy(GT[:, gb, tch * FD : (tch + 1) * FD],
                                       pss[gb][:])

                if sg == 0:
                    emit_GT(sg)
                    emit_V(sg)
                else:
                    emit_V(sg)
                    emit_GT(sg)

            # ---- scores S[s,t] + softmax over t (free axis) ----
            Etile = pp.tile([P, NT, T], F16, tag="E")
            for st in range(NT):
                pss = [
                    psp.tile([P, FD], F32, tag="mm", name=f"ps_s{st}_{tch}")
                    for tch in range(NC_T)
                ]
                for tch in range(NC_T):
                    for hb in range(NH):
                        nc.tensor.matmul(
                            pss[tch][:],
                            tokT[:, hb, st * P : (st + 1) * P],
                            GT[:, hb, tch * FD : (tch + 1) * FD],
                            start=(hb == 0),
                            stop=(hb == NH - 1),
                        )
                mx4 = stp.tile([P, NC_T], F32, tag="mx4")
                for tch in range(NC_T):
                    nc.vector.reduce_max(
                        mx4[:, tch : tch + 1], pss[tch][:], axis=AX.X
                    )
                nmx = stp.tile([P, 1], F32, tag="nmx")
                nc.vector.reduce_max(nmx[:], mx4[:], axis=AX.X, negate=True)
                racc = stp.tile([P, NC_T], F32, tag="racc")
                for tch in range(NC_T):
                    nc.scalar.activation(
                        Etile[:, st, tch * FD : (tch + 1) * FD],
                        pss[tch][:],
                        AF.Exp,
                        bias=nmx[:],
                        accum_out=racc[:, tch : tch + 1],
                    )
                rsum = stp.tile([P, 1], F32, tag="rsum")
                nc.vector.reduce_sum(rsum[:], racc[:], axis=AX.X)
                rinv = stp.tile([P, 1], F32, tag="rinv")
                nc.vector.reciprocal(rinv[:], rsum[:])
                # Fold 1/rowsum into V rows (rowsum is per-s, V is s-major).
                nc.vector.tensor_scalar_mul(V[:, st, :], V[:, st, :], rinv[:])

            # ---- context: ctx[t,h] = sum_s E[s,t] * V'[s,h] ----
            for tt in range(NT):
                if tt < NT - 1:
                    ps = psp.tile([P, FD], F32, tag="mm", name=f"ps_c{tt}")
                    for st in range(NT):
                        nc.tensor.matmul(
                            ps[:],
                            Etile[:, st, tt * P : (tt + 1) * P],
                            V[:, st, :],
                            start=(st == 0),
                            stop=(st == NT - 1),
                        )
                    ot = osp.tile([P, H], F32, tag="ostage", name=f"ost{tt}")
                    nc.vector.tensor_copy(ot[:], ps[:])
                    nc.sync.dma_start(out_tiled[tt], ot[:])
                else:
                    # Last tile: split the tail chain by partition halves
                    # across two engine pairs (DVE evac + Sync push for
                    # rows 0-63, ScalarE evac + Scalar-queue push for rows
                    # 64-127) so the evacs and the two 128KB transfers
                    # run in parallel instead of one serial 256KB chain.
                    ps = psp.tile([P, FD], F32, tag="mm", name=f"ps_c{tt}")
                    for st in range(NT):
                        nc.tensor.matmul(
                            ps[:],
                            Etile[:, st, tt * P : (tt + 1) * P],
                            V[:, st, :],
                            start=(st == 0),
                            stop=(st == NT - 1),
                        )
                    HP = P // 2
                    ot = osp.tile([P, H], F32, tag="ostage", name=f"ost{tt}")
                    nc.vector.tensor_copy(ot[0:HP, :], ps[0:HP, :])
                    nc.sync.dma_start(out_tiled[tt][0:HP, :], ot[0:HP, :])
                    nc.scalar.copy(ot[HP:P, :], ps[HP:P, :])
                    nc.scalar.dma_start(out_tiled[tt][HP:P, :], ot[HP:P, :])

    nc.compile()
    return nc


_NC = None


def _get_nc():
    global _NC
    if _NC is None:
        _NC = build()
    return _NC


def _run(inputs, trace=False, **kwargs):
    tokens = np.ascontiguousarray(inputs["tokens"], dtype=np.float32)
    Wq = np.ascontiguousarray(inputs["Wq"], dtype=np.float32)
    Wk = np.ascontiguousarray(inputs["Wk"], dtype=np.float32)
    Wv = np.ascontiguousarray(inputs["Wv"], dtype=np.float32)
    assert tokens.shape == (B, T, H)
    nc = _get_nc()
    in_maps = [
        {"tokens": tokens[i], "Wq": Wq, "Wk": Wk, "Wv": Wv} for i in range(N_CORES)
    ]
    res = run_bass_kernel_spmd(
        nc, in_maps, core_ids=list(range(N_CORES)), trace=trace, **kwargs
    )
    out = np.stack([res.results[i]["out"] for i in range(N_CORES)], axis=0)
    return out.astype(np.float32), res


def kernel(**inputs) -> np.ndarray:
    out, _ = _run(inputs)
    return out



# revision 11
# speedup vs baseline: 1.0570x; 1.0570x over previous
"""Trainium2 Bass kernel for batched attention with softmax over the query axis.

Reference computation (per batch element b):
    Q = tokens @ Wq; K = tokens @ Wk; V = tokens @ Wv
    S = Q @ K.T                [T(t), T(s)]
    A = softmax(S, axis=t)     (normalizes over the *query* axis per key column)
    out = A @ V                [T, H]

Sharding: pure data parallelism - B=8 batch elements, one per NeuronCore.
The softmax couples queries only within a batch element, so no collectives.

Per-core implementation (fp16 matmul operands, fp32 PSUM accumulation):
  - W_qk = Wq @ Wk.T is built on-chip (weight-only work that overlaps the
    token DMA), so scores need one projection G = tokens @ W_qk instead of
    separate Q and K: S = G @ tokens.T.
  - All input DMAs flow through one ring of [128,512] f32 chunk tiles on the
    Sync queue in dependency-priority order (wq, wk, tok stage 0, wv, tok
    stages 1-3); slot reuse orders later transfers behind earlier consumers.
  - Weights are transposed on the PE in f32 straight off the DMA stage (PE is
    DMA-paced idle there; the batched DVE evacuation doubles as the f16 cast,
    keeping the DVE off the W_qk critical path). Tokens are DVE-cast to f16
    then PE-transposed (tokT) with one batched DVE evac per t-tile.
  - GT   [g%128, g//128, t] via lhsT=W_qk rhs=tokT.
  - V    [s%128, s//128, h] via lhsT=tokT rhs=Wv; evacuated by ScalarE so the
    next stage's t16 casts stay at the DVE queue head (no stage-boundary PE
    bubble). Stage 0 runs GT before V (wv arrives later); stages 1-3 run V
    first (V tile st needs only the first tokT evacuation of the stage).
  - S_st [s%128, t] via lhsT=tokT rhs=GT -> softmax over t is a free-axis
    reduction: max (DVE), exp (ScalarE, accum_out produces row sums).
  - 1/rowsum is folded into V rows, so the unnormalized exp tile E feeds the
    context matmul: ctx[t,h] via lhsT=E rhs=V' accumulated over s.
The matmul stream runs gapless at the 216ns/512-free cadence from the end of
prep to the last context matmul; engine assignment is balanced so DVE/ScalarE
softmax work sustains that cadence.

NOTE: several seemingly-better variants deterministically flip the chip to a
20% slower clock for the whole NEFF (extra ScalarE casts in prep, fp8
DoubleRow matmuls, DMA-crossbar transposes). Compare runs by matmul cadence
(216ns fast vs 259ns slow), not wall time alone, before believing a result.
"""

import numpy as np

import concourse.bass as bass
import concourse.bacc as bacc
import concourse.tile as tile
from concourse import mybir
from concourse.bass_utils import run_bass_kernel_spmd
from concourse.masks import make_identity

B, T, H, E = 8, 2048, 512, 512
P = 128
NT = T // P      # 16 tiles along t / s
NH = H // P      # 4 tiles along h
FD = 512         # matmul moving free dim (one fp32 PSUM bank)
NC_T = T // FD   # 4 free-dim chunks along t
NST = T // FD    # 4 token stage groups (4 t-tiles each)

F32 = mybir.dt.float32
F16 = mybir.dt.float16
AX = mybir.AxisListType
AF = mybir.ActivationFunctionType

N_CORES = 8


def build():
    nc = bacc.Bacc()
    tok_d = nc.declare_dram_parameter("tokens", [T, H], F32, isOutput=False)
    wq_d = nc.declare_dram_parameter("Wq", [H, E], F32, isOutput=False)
    wk_d = nc.declare_dram_parameter("Wk", [H, E], F32, isOutput=False)
    wv_d = nc.declare_dram_parameter("Wv", [H, H], F32, isOutput=False)
    out_d = nc.declare_dram_parameter("out", [T, H], F32, isOutput=True)

    # [p, tt, h]: partition = t%128, stage groups of 4 t-tiles -> 1MB DMAs
    tok_staged = tok_d.rearrange("(sg tt p) h -> sg p tt h", p=P, tt=NT // NST)
    out_tiled = out_d.rearrange("(tt p) h -> tt p h", p=P)

    with tile.TileContext(nc) as tc:
        with (
            tc.tile_pool(name="persist", bufs=1) as pp,
            tc.tile_pool(name="stage", bufs=2) as sp,
            tc.tile_pool(name="w16", bufs=4) as wp,
            tc.tile_pool(name="t16", bufs=6) as tp,
            tc.tile_pool(name="ostage", bufs=3) as osp,
            tc.tile_pool(name="stats", bufs=4) as stp,
            tc.tile_pool(name="psum", bufs=8, space=bass.MemorySpace.PSUM) as psp,
        ):
            ident = pp.tile([P, P], F16, tag="ident")
            make_identity(nc, ident[:])
            ident32 = pp.tile([P, P], F32, tag="ident32")
            make_identity(nc, ident32[:])

            # ---- HAM warm-up: the PE clock gate defaults to K=4/8
            # (1.2 GHz) and only reaches 8/8 after ~3.4us of sustained
            # REAL matmul activity (transpose-mode does not count).
            # Without this the whole prep phase runs at half clock
            # (trace: HAM K=8 only at ts~19us). Dummy ident x ident
            # matmuls bridge the DMA-bound head: a small front batch
            # (ident ready ~7.8us -> first wk chunk ~9.0us), plus 4
            # dummies after each wk-chunk transpose group to keep the
            # HAM activity window fed until Wqk's dense matmuls start.
            # Counts are sized to never head-of-line-block real work.
            ps_warm = psp.tile([P, P], F32, tag="mm", name="ps_warm")

            def warmup(n):
                for _ in range(n):
                    nc.tensor.matmul(ps_warm[:], ident[:], ident[:],
                                     start=True, stop=True)

            warmup(12)

            # ---- input DMA pushes (Sync queue) through ONE ring of 8
            # [128,512] f32 chunk tiles. Slot reuse makes each later DMA wait
            # until the earlier chunk in that slot has been consumed, so the
            # weight chunks get near-exclusive HBM bandwidth up front instead
            # of round-robin sharing with 4MB of token traffic.
            def chunk(name):
                return sp.tile([P, FD], F32, tag="st32", bufs=8, name=name)

            wstages = {"wq": [], "wk": []}
            wtiled = {
                "wq": wq_d.rearrange("(hh p) e -> hh p e", p=P),
                "wk": wk_d.rearrange("(hh p) e -> hh p e", p=P),
            }
            for name in ("wk", "wq"):
                for hh in range(NH):
                    ws = chunk(f"wst_{name}{hh}")
                    nc.sync.dma_start(ws[:], wtiled[name][hh])
                    wstages[name].append(ws)

            tok_chunked = tok_d.rearrange("(sg tt p) h -> sg tt p h", p=P,
                                          tt=NT // NST)
            tchunks = [[None] * (NT // NST) for _ in range(NST)]
            for ti in range(NT // NST):
                tc0 = chunk(f"tok0_{ti}")
                nc.sync.dma_start(tc0[:], tok_chunked[0][ti])
                tchunks[0][ti] = tc0

            wv_tiled = wv_d.rearrange("(hh p) e -> hh p e", p=P)
            wvchunks = []
            for hh in range(NH):
                wc = chunk(f"wv{hh}")
                nc.sync.dma_start(wc[:], wv_tiled[hh])
                wvchunks.append(wc)

            for sg in range(1, NST):
                for ti in range(NT // NST):
                    tcx = chunk(f"tok{sg}_{ti}")
                    nc.sync.dma_start(tcx[:], tok_chunked[sg][ti])
                    tchunks[sg][ti] = tcx

            # ---- Wq/Wk: f32 PE transpose straight off the DMA stage (PE
            # is DMA-paced here; the batched DVE evacuation doubles as
            # the f16 cast). Each W_qk[hb] matmul group is emitted right
            # after its wq-chunk transpose so dense real matmuls start as
            # soon as wq chunks land; wk groups get dummy-matmul filler
            # to keep the HAM window busy during DMA waits.
            wT16 = {
                "wq": pp.tile([P, NH, E], F16, tag="wqT", name="wT_wq"),
                "wk": pp.tile([P, NH, E], F16, tag="wkT", name="wT_wk"),
            }
            Wqk = pp.tile([P, NH, H], F16, tag="Wqk")
            for name in ("wk", "wq"):
                for hh in range(NH):
                    # f32 transpose straight off the stage: PE is DMA-paced
                    # idle there, and it keeps the DVE off the W_qk
                    # critical path (one batched evac per chunk is all).
                    ps_tr = psp.tile([P, NH, P], F32, tag="mm",
                                     name=f"tr_{name}{hh}")
                    for eb in range(NH):
                        nc.tensor.transpose(
                            ps_tr[:, eb],
                            wstages[name][hh][:, eb * P : (eb + 1) * P],
                            ident32[:],
                        )
                    nc.vector.tensor_copy(
                        wT16[name][:, :, hh * P : (hh + 1) * P], ps_tr[:]
                    )
                    # fill the DMA wait for the next chunk with real
                    # (HAM-visible) matmul activity; transposes alone do
                    # not count as PE-busy for the HAM clock gate
                    warmup(4 if name == "wk" else 2)

            # ---- W_qk = Wq @ Wk.T : [h%128, hb, h'] fp16 ----
            # Emitted batched after all transposes: interleaving it per
            # wq chunk was measured slower (the PE then waits out the
            # PSUM->SBUF evac latency once per chunk instead of once).
            for hb in range(NH):
                ps = psp.tile([P, FD], F32, tag="mm", name=f"ps_wqk{hb}")
                for eb in range(NH):
                    nc.tensor.matmul(
                        ps[:],
                        wT16["wq"][:, eb, hb * P : (hb + 1) * P],
                        wT16["wk"][:, eb, :],
                        start=(eb == 0),
                        stop=(eb == NH - 1),
                    )
                nc.scalar.copy(Wqk[:, hb, :], ps[:])

            # Token stage 0 casts early in the DVE queue (GpSimd measured
            # 4.3x slower per cast, so they stay on the DVE).
            t16s0 = []
            for ti in range(NT // NST):
                t16 = tp.tile([P, H], F16, tag="t16", name=f"t16_{ti}")
                nc.vector.tensor_copy(t16[:], tchunks[0][ti][:])
                t16s0.append(t16)

            wv16 = pp.tile([P, NH, E], F16, tag="wv16")

            # ---- per stage: transposes -> GT chunk -> V tiles ----
            tokT = pp.tile([P, NH, T], F16, tag="tokT")
            GT = pp.tile([P, NH, T], F16, tag="GT")
            V = pp.tile([P, NT, H], F16, tag="V")
            for sg in range(NST):
                # t16 casts for this stage (stage 0 pre-cast above).
                if sg == 0:
                    t16s = t16s0
                else:
                    t16s = []
                    for ti in range(NT // NST):
                        t16 = tp.tile([P, H], F16, tag="t16",
                                      name=f"t16_{sg * (NT // NST) + ti}")
                        nc.vector.tensor_copy(t16[:], tchunks[sg][ti][:])
                        t16s.append(t16)
                # Transposes for two t-tiles share one PSUM tile so the
                # stage needs 2 DVE evacuations instead of 4 (the evac
                # chain gates GT[sg]'s rhs).
                for pi in range(NT // NST // 2):
                    ps_tr = psp.tile([P, NH, 2 * P], F16, tag="mm",
                                     name=f"trt{sg}_{pi}")
                    for k in range(2):
                        ti = pi * 2 + k
                        for ht in range(NH):
                            nc.tensor.transpose(
                                ps_tr[:, ht, k * P : (k + 1) * P],
                                t16s[ti][:, ht * P : (ht + 1) * P],
                                ident[:],
                            )
                    base = sg * FD + pi * 2 * P
                    nc.vector.tensor_copy(
                        tokT[:, :, base : base + 2 * P], ps_tr[:]
                    )
                if sg == 0:
                    # wv casts after stage-0 tokT evacs in the DVE queue
                    # so GT[0] isn't stuck behind them.
                    for hh in range(NH):
                        nc.vector.tensor_copy(wv16[:, hh], wvchunks[hh][:])

                def emit_V(sg):
                    for st in range(sg * NC_T, (sg + 1) * NC_T):
                        ps = psp.tile([P, FD], F32, tag="mm", name=f"ps_v{st}")
                        for ht in range(NH):
                            nc.tensor.matmul(
                                ps[:],
                                tokT[:, ht, st * P : (st + 1) * P],
                                wv16[:, ht, :],
                                start=(ht == 0),
                                stop=(ht == NH - 1),
                            )
                        nc.scalar.copy(V[:, st, :], ps[:])

                def emit_GT(sg):
                    # hb-outer: the first 12 matmuls need only Wqk[0..2],
                    # so GT[0] isn't stalled on Wqk[3]'s ScalarE evac.
                    tch = sg
                    pss = [
                        psp.tile([P, FD], F32, tag="mm",
                                 name=f"ps_g{gb}_{tch}")
                        for gb in range(NH)
                    ]
                    for hb in range(NH):
                        for gb in range(NH):
                            nc.tensor.matmul(
                                pss[gb][:],
                                Wqk[:, hb, gb * P : (gb + 1) * P],
                                tokT[:, hb, tch * FD : (tch + 1) * FD],
                                start=(hb == 0),
                                stop=(hb == NH - 1),
                            )
                    for gb in range(NH):
                        nc.scalar.copy(GT[:, gb, tch * FD : (tch + 1) * FD],
                                       pss[gb][:])

                if sg == 0:
                    emit_GT(sg)
                    emit_V(sg)
                else:
                    emit_V(sg)
                    emit_GT(sg)

            # ---- scores S[s,t] + softmax over t (free axis) ----
            Etile = pp.tile([P, NT, T], F16, tag="E")
            for st in range(NT):
                pss = [
                    psp.tile([P, FD], F32, tag="mm", name=f"ps_s{st}_{tch}")
                    for tch in range(NC_T)
                ]
                for tch in range(NC_T):
                    for hb in range(NH):
                        nc.tensor.matmul(
                            pss[tch][:],
                            tokT[:, hb, st * P : (st + 1) * P],
                            GT[:, hb, tch * FD : (tch + 1) * FD],
                            start=(hb == 0),
                            stop=(hb == NH - 1),
                        )
                mx4 = stp.tile([P, NC_T], F32, tag="mx4")
                for tch in range(NC_T):
                    nc.vector.reduce_max(
                        mx4[:, tch : tch + 1], pss[tch][:], axis=AX.X
                    )
                nmx = stp.tile([P, 1], F32, tag="nmx")
                nc.vector.reduce_max(nmx[:], mx4[:], axis=AX.X, negate=True)
                racc = stp.tile([P, NC_T], F32, tag="racc")
                for tch in range(NC_T):
                    nc.scalar.activation(
                        Etile[:, st, tch * FD : (tch + 1) * FD],
                        pss[tch][:],
                        AF.Exp,
                        bias=nmx[:],
                        accum_out=racc[:, tch : tch + 1],
                    )
                rsum = stp.tile([P, 1], F32, tag="rsum")
                nc.vector.reduce_sum(rsum[:], racc[:], axis=AX.X)
                rinv = stp.tile([P, 1], F32, tag="rinv")
                nc.vector.reciprocal(rinv[:], rsum[:])
                # Fold 1/rowsum into V rows (rowsum is per-s, V is s-major).
                nc.vector.tensor_scalar_mul(V[:, st, :], V[:, st, :], rinv[:])

            # ---- context: ctx[t,h] = sum_s E[s,t] * V'[s,h] ----
            for tt in range(NT):
                if tt < NT - 1:
                    ps = psp.tile([P, FD], F32, tag="mm", name=f"ps_c{tt}")
                    for st in range(NT):
                        nc.tensor.matmul(
                            ps[:],
                            Etile[:, st, tt * P : (tt + 1) * P],
                            V[:, st, :],
                            start=(st == 0),
                            stop=(st == NT - 1),
                        )
                    ot = osp.tile([P, H], F32, tag="ostage", name=f"ost{tt}")
                    nc.vector.tensor_copy(ot[:], ps[:])
                    nc.sync.dma_start(out_tiled[tt], ot[:])
                else:
                    # Last tile: split the tail chain by partition halves
                    # across two engine pairs (DVE evac + Sync push for
                    # rows 0-63, ScalarE evac + Scalar-queue push for rows
                    # 64-127) so the evacs and the two 128KB transfers
                    # run in parallel instead of one serial 256KB chain.
                    ps = psp.tile([P, FD], F32, tag="mm", name=f"ps_c{tt}")
                    for st in range(NT):
                        nc.tensor.matmul(
                            ps[:],
                            Etile[:, st, tt * P : (tt + 1) * P],
                            V[:, st, :],
                            start=(st == 0),
                            stop=(st == NT - 1),
                        )
                    HP = P // 2
                    ot = osp.tile([P, H], F32, tag="ostage", name=f"ost{tt}")
                    nc.vector.tensor_copy(ot[0:HP, :], ps[0:HP, :])
                    nc.sync.dma_start(out_tiled[tt][0:HP, :], ot[0:HP, :])
                    nc.scalar.copy(ot[HP:P, :], ps[HP:P, :])
                    nc.scalar.dma_start(out_tiled[tt][HP:P, :], ot[HP:P, :])

    nc.compile()
    return nc


_NC = None


def _get_nc():
    global _NC
    if _NC is None:
        _NC = build()
    return _NC


def _run(inputs, trace=False, **kwargs):
    tokens = np.ascontiguousarray(inputs["tokens"], dtype=np.float32)
    Wq = np.ascontiguousarray(inputs["Wq"], dtype=np.float32)
    Wk = np.ascontiguousarray(inputs["Wk"], dtype=np.float32)
    Wv = np.ascontiguousarray(inputs["Wv"], dtype=np.float32)
    assert tokens.shape == (B, T, H)
    nc = _get_nc()
    in_maps = [
        {"tokens": tokens[i], "Wq": Wq, "Wk": Wk, "Wv": Wv} for i in range(N_CORES)
    ]
    res = run_bass_kernel_spmd(
        nc, in_maps, core_ids=list(range(N_CORES)), trace=trace, **kwargs
    )
    out = np.stack([res.results[i]["out"] for i in range(N_CORES)], axis=0)
    return out.astype(np.float32), res


def kernel(**inputs) -> np.ndarray:
    out, _ = _run(inputs)
    return out



# revision 12
# speedup vs baseline: 1.0735x; 1.0156x over previous
"""Trainium2 Bass kernel for batched attention with softmax over the query axis.

Reference computation (per batch element b):
    Q = tokens @ Wq; K = tokens @ Wk; V = tokens @ Wv
    S = Q @ K.T                [T(t), T(s)]
    A = softmax(S, axis=t)     (normalizes over the *query* axis per key column)
    out = A @ V                [T, H]

Sharding: pure data parallelism - B=8 batch elements, one per NeuronCore.
The softmax couples queries only within a batch element, so no collectives.

Per-core implementation (fp16 matmul operands, fp32 PSUM accumulation):
  - W_qk = Wq @ Wk.T is built on-chip (weight-only work that overlaps the
    token DMA), so scores need one projection G = tokens @ W_qk instead of
    separate Q and K: S = G @ tokens.T.
  - All input DMAs flow through one ring of [128,512] f32 chunk tiles on the
    Sync queue in dependency-priority order (wq, wk, tok stage 0, wv, tok
    stages 1-3); slot reuse orders later transfers behind earlier consumers.
  - Weights are transposed on the PE in f32 straight off the DMA stage (PE is
    DMA-paced idle there; the batched DVE evacuation doubles as the f16 cast,
    keeping the DVE off the W_qk critical path). Tokens are DVE-cast to f16
    then PE-transposed (tokT) with one batched DVE evac per t-tile.
  - GT   [g%128, g//128, t] via lhsT=W_qk rhs=tokT.
  - V    [s%128, s//128, h] via lhsT=tokT rhs=Wv; evacuated by ScalarE so the
    next stage's t16 casts stay at the DVE queue head (no stage-boundary PE
    bubble). Stage 0 runs GT before V (wv arrives later); stages 1-3 run V
    first (V tile st needs only the first tokT evacuation of the stage).
  - S_st [s%128, t] via lhsT=tokT rhs=GT -> softmax over t is a free-axis
    reduction: max (DVE), exp (ScalarE, accum_out produces row sums).
  - 1/rowsum is folded into V rows, so the unnormalized exp tile E feeds the
    context matmul: ctx[t,h] via lhsT=E rhs=V' accumulated over s.
The matmul stream runs gapless at the 216ns/512-free cadence from the end of
prep to the last context matmul; engine assignment is balanced so DVE/ScalarE
softmax work sustains that cadence.

NOTE: several seemingly-better variants deterministically flip the chip to a
20% slower clock for the whole NEFF (extra ScalarE casts in prep, fp8
DoubleRow matmuls, DMA-crossbar transposes). Compare runs by matmul cadence
(216ns fast vs 259ns slow), not wall time alone, before believing a result.
"""

import numpy as np

import concourse.bass as bass
import concourse.bacc as bacc
import concourse.tile as tile
from concourse import mybir
from concourse.bass_utils import run_bass_kernel_spmd
from concourse.masks import make_identity

B, T, H, E = 8, 2048, 512, 512
P = 128
NT = T // P      # 16 tiles along t / s
NH = H // P      # 4 tiles along h
FD = 512         # matmul moving free dim (one fp32 PSUM bank)
NC_T = T // FD   # 4 free-dim chunks along t
NST = T // FD    # 4 token stage groups (4 t-tiles each)

F32 = mybir.dt.float32
F16 = mybir.dt.float16
AX = mybir.AxisListType
AF = mybir.ActivationFunctionType

N_CORES = 8


def build():
    nc = bacc.Bacc()
    tok_d = nc.declare_dram_parameter("tokens", [T, H], F32, isOutput=False)
    wq_d = nc.declare_dram_parameter("Wq", [H, E], F32, isOutput=False)
    wk_d = nc.declare_dram_parameter("Wk", [H, E], F32, isOutput=False)
    wv_d = nc.declare_dram_parameter("Wv", [H, H], F32, isOutput=False)
    out_d = nc.declare_dram_parameter("out", [T, H], F32, isOutput=True)

    # [p, tt, h]: partition = t%128, stage groups of 4 t-tiles -> 1MB DMAs
    tok_staged = tok_d.rearrange("(sg tt p) h -> sg p tt h", p=P, tt=NT // NST)
    out_tiled = out_d.rearrange("(tt p) h -> tt p h", p=P)

    with tile.TileContext(nc) as tc:
        with (
            tc.tile_pool(name="persist", bufs=1) as pp,
            tc.tile_pool(name="stage", bufs=2) as sp,
            tc.tile_pool(name="w16", bufs=4) as wp,
            tc.tile_pool(name="t16", bufs=6) as tp,
            tc.tile_pool(name="ostage", bufs=3) as osp,
            tc.tile_pool(name="stats", bufs=4) as stp,
            tc.tile_pool(name="psum", bufs=8, space=bass.MemorySpace.PSUM) as psp,
        ):
            ident = pp.tile([P, P], F16, tag="ident")
            make_identity(nc, ident[:])
            ident32 = pp.tile([P, P], F32, tag="ident32")
            make_identity(nc, ident32[:])

            # ---- HAM warm-up: the PE clock gate defaults to K=4/8
            # (1.2 GHz) and only reaches 8/8 after ~3.4us of sustained
            # REAL matmul activity (transpose-mode does not count).
            # Without this the whole prep phase runs at half clock
            # (trace: HAM K=8 only at ts~19us). Dummy ident x ident
            # matmuls bridge the DMA-bound head: a small front batch
            # (ident ready ~7.8us -> first wk chunk ~9.0us), plus 4
            # dummies after each wk-chunk transpose group to keep the
            # HAM activity window fed until Wqk's dense matmuls start.
            # Counts are sized to never head-of-line-block real work.
            ps_warm = psp.tile([P, P], F32, tag="mm", name="ps_warm")

            def warmup(n):
                for _ in range(n):
                    nc.tensor.matmul(ps_warm[:], ident[:], ident[:],
                                     start=True, stop=True)

            warmup(12)

            # ---- input DMA pushes (Sync queue) through ONE ring of 8
            # [128,512] f32 chunk tiles. Slot reuse makes each later DMA wait
            # until the earlier chunk in that slot has been consumed, so the
            # weight chunks get near-exclusive HBM bandwidth up front instead
            # of round-robin sharing with 4MB of token traffic.
            def chunk(name):
                return sp.tile([P, FD], F32, tag="st32", bufs=8, name=name)

            wstages = {"wq": [], "wk": []}
            wtiled = {
                "wq": wq_d.rearrange("(hh p) e -> hh p e", p=P),
                "wk": wk_d.rearrange("(hh p) e -> hh p e", p=P),
            }
            for name in ("wk", "wq"):
                for hh in range(NH):
                    ws = chunk(f"wst_{name}{hh}")
                    nc.sync.dma_start(ws[:], wtiled[name][hh])
                    wstages[name].append(ws)

            tok_chunked = tok_d.rearrange("(sg tt p) h -> sg tt p h", p=P,
                                          tt=NT // NST)
            tchunks = [[None] * (NT // NST) for _ in range(NST)]
            for ti in range(NT // NST):
                tc0 = chunk(f"tok0_{ti}")
                nc.sync.dma_start(tc0[:], tok_chunked[0][ti])
                tchunks[0][ti] = tc0

            wv_tiled = wv_d.rearrange("(hh p) e -> hh p e", p=P)
            wvchunks = []
            for hh in range(NH):
                wc = chunk(f"wv{hh}")
                nc.sync.dma_start(wc[:], wv_tiled[hh])
                wvchunks.append(wc)

            for sg in range(1, NST):
                for ti in range(NT // NST):
                    tcx = chunk(f"tok{sg}_{ti}")
                    nc.sync.dma_start(tcx[:], tok_chunked[sg][ti])
                    tchunks[sg][ti] = tcx

            # ---- Wq/Wk: f32 PE transpose straight off the DMA stage (PE
            # is DMA-paced here; the batched DVE evacuation doubles as
            # the f16 cast). Each W_qk[hb] matmul group is emitted right
            # after its wq-chunk transpose so dense real matmuls start as
            # soon as wq chunks land; wk groups get dummy-matmul filler
            # to keep the HAM window busy during DMA waits.
            wT16 = {
                "wq": pp.tile([P, NH, E], F16, tag="wqT", name="wT_wq"),
                "wk": pp.tile([P, NH, E], F16, tag="wkT", name="wT_wk"),
            }
            Wqk = pp.tile([P, NH, H], F16, tag="Wqk")
            for name in ("wk", "wq"):
                for hh in range(NH):
                    # f32 transpose straight off the stage: PE is DMA-paced
                    # idle there, and it keeps the DVE off the W_qk
                    # critical path (one batched evac per chunk is all).
                    ps_tr = psp.tile([P, NH, P], F32, tag="mm",
                                     name=f"tr_{name}{hh}")
                    for eb in range(NH):
                        nc.tensor.transpose(
                            ps_tr[:, eb],
                            wstages[name][hh][:, eb * P : (eb + 1) * P],
                            ident32[:],
                        )
                    nc.vector.tensor_copy(
                        wT16[name][:, :, hh * P : (hh + 1) * P], ps_tr[:]
                    )
                    # fill the DMA wait for the next chunk with real
                    # (HAM-visible) matmul activity; transposes alone do
                    # not count as PE-busy for the HAM clock gate
                    warmup(4 if name == "wk" else 2)

            # ---- W_qk = Wq @ Wk.T : [h%128, hb, h'] fp16 ----
            # Emitted batched after all transposes: interleaving it per
            # wq chunk was measured slower (the PE then waits out the
            # PSUM->SBUF evac latency once per chunk instead of once).
            for hb in range(NH):
                ps = psp.tile([P, FD], F32, tag="mm", name=f"ps_wqk{hb}")
                for eb in range(NH):
                    nc.tensor.matmul(
                        ps[:],
                        wT16["wq"][:, eb, hb * P : (hb + 1) * P],
                        wT16["wk"][:, eb, :],
                        start=(eb == 0),
                        stop=(eb == NH - 1),
                    )
                nc.scalar.copy(Wqk[:, hb, :], ps[:])

            # Token stage 0 casts early in the DVE queue (GpSimd measured
            # 4.3x slower per cast, so they stay on the DVE).
            t16s0 = []
            for ti in range(NT // NST):
                t16 = tp.tile([P, H], F16, tag="t16", name=f"t16_{ti}")
                nc.vector.tensor_copy(t16[:], tchunks[0][ti][:])
                t16s0.append(t16)

            wv16 = pp.tile([P, NH, E], F16, tag="wv16")

            # ---- per stage: transposes -> GT chunk -> V tiles ----
            tokT = pp.tile([P, NH, T], F16, tag="tokT")
            GT = pp.tile([P, NH, T], F16, tag="GT")
            V = pp.tile([P, NT, H], F16, tag="V")
            for sg in range(NST):
                # t16 casts for this stage (stage 0 pre-cast above).
                if sg == 0:
                    t16s = t16s0
                else:
                    t16s = []
                    for ti in range(NT // NST):
                        t16 = tp.tile([P, H], F16, tag="t16",
                                      name=f"t16_{sg * (NT // NST) + ti}")
                        nc.vector.tensor_copy(t16[:], tchunks[sg][ti][:])
                        t16s.append(t16)
                for ti in range(NT // NST):
                    tt = sg * (NT // NST) + ti
                    ps_tr = psp.tile([P, NH, P], F16, tag="mm", name=f"trt{tt}")
                    for ht in range(NH):
                        nc.tensor.transpose(
                            ps_tr[:, ht],
                            t16s[ti][:, ht * P : (ht + 1) * P],
                            ident[:],
                        )
                    nc.vector.tensor_copy(
                        tokT[:, :, tt * P : (tt + 1) * P], ps_tr[:]
                    )
                if sg == 0:
                    # wv casts after stage-0 tokT evacs in the DVE queue
                    # so GT[0] isn't stuck behind them.
                    for hh in range(NH):
                        nc.vector.tensor_copy(wv16[:, hh], wvchunks[hh][:])

                def emit_V(sg):
                    for st in range(sg * NC_T, (sg + 1) * NC_T):
                        ps = psp.tile([P, FD], F32, tag="mm", name=f"ps_v{st}")
                        for ht in range(NH):
                            nc.tensor.matmul(
                                ps[:],
                                tokT[:, ht, st * P : (st + 1) * P],
                                wv16[:, ht, :],
                                start=(ht == 0),
                                stop=(ht == NH - 1),
                            )
                        nc.scalar.copy(V[:, st, :], ps[:])

                def emit_GT(sg):
                    tch = sg
                    for gb in range(NH):
                        ps = psp.tile([P, FD], F32, tag="mm",
                                      name=f"ps_g{gb}_{tch}")
                        for hb in range(NH):
                            nc.tensor.matmul(
                                ps[:],
                                Wqk[:, hb, gb * P : (gb + 1) * P],
                                tokT[:, hb, tch * FD : (tch + 1) * FD],
                                start=(hb == 0),
                                stop=(hb == NH - 1),
                            )
                        nc.scalar.copy(GT[:, gb, tch * FD : (tch + 1) * FD],
                                       ps[:])

                if sg == 0:
                    emit_GT(sg)
                    emit_V(sg)
                else:
                    emit_V(sg)
                    emit_GT(sg)

            # ---- scores S[s,t] + softmax over t (free axis) ----
            Etile = pp.tile([P, NT, T], F16, tag="E")
            for st in range(NT):
                pss = [
                    psp.tile([P, FD], F32, tag="mm", name=f"ps_s{st}_{tch}")
                    for tch in range(NC_T)
                ]
                for tch in range(NC_T):
                    for hb in range(NH):
                        nc.tensor.matmul(
                            pss[tch][:],
                            tokT[:, hb, st * P : (st + 1) * P],
                            GT[:, hb, tch * FD : (tch + 1) * FD],
                            start=(hb == 0),
                            stop=(hb == NH - 1),
                        )
                mx4 = stp.tile([P, NC_T], F32, tag="mx4")
                for tch in range(NC_T):
                    nc.vector.reduce_max(
                        mx4[:, tch : tch + 1], pss[tch][:], axis=AX.X
                    )
                nmx = stp.tile([P, 1], F32, tag="nmx")
                nc.vector.reduce_max(nmx[:], mx4[:], axis=AX.X, negate=True)
                racc = stp.tile([P, NC_T], F32, tag="racc")
                for tch in range(NC_T):
                    nc.scalar.activation(
                        Etile[:, st, tch * FD : (tch + 1) * FD],
                        pss[tch][:],
                        AF.Exp,
                        bias=nmx[:],
                        accum_out=racc[:, tch : tch + 1],
                    )
                rsum = stp.tile([P, 1], F32, tag="rsum")
                nc.vector.reduce_sum(rsum[:], racc[:], axis=AX.X)
                rinv = stp.tile([P, 1], F32, tag="rinv")
                nc.vector.reciprocal(rinv[:], rsum[:])
                # Fold 1/rowsum into V rows (rowsum is per-s, V is s-major).
                nc.vector.tensor_scalar_mul(V[:, st, :], V[:, st, :], rinv[:])

            # ---- context: ctx[t,h] = sum_s E[s,t] * V'[s,h] ----
            for tt in range(NT):
                if tt < NT - 1:
                    ps = psp.tile([P, FD], F32, tag="mm", name=f"ps_c{tt}")
                    for st in range(NT):
                        nc.tensor.matmul(
                            ps[:],
                            Etile[:, st, tt * P : (tt + 1) * P],
                            V[:, st, :],
                            start=(st == 0),
                            stop=(st == NT - 1),
                        )
                    ot = osp.tile([P, H], F32, tag="ostage", name=f"ost{tt}")
                    nc.vector.tensor_copy(ot[:], ps[:])
                    nc.sync.dma_start(out_tiled[tt], ot[:])
                else:
                    # Last tile: two FD-256 halves sharing each st's
                    # stationary operand (one LDW, two matmuls at the same
                    # ~110ns cadence), so half the output is evacuated and
                    # pushed while the other half finishes accumulating.
                    HF = FD // 2
                    pss = [
                        psp.tile([P, HF], F32, tag="mm", name=f"ps_c{tt}_{c}")
                        for c in range(2)
                    ]
                    for st in range(NT):
                        for c in range(2):
                            nc.tensor.matmul(
                                pss[c][:],
                                Etile[:, st, tt * P : (tt + 1) * P],
                                V[:, st, c * HF : (c + 1) * HF],
                                start=(st == 0),
                                stop=(st == NT - 1),
                            )
                    for c in range(2):
                        ot = osp.tile([P, HF], F32, tag="ostrip",
                                      name=f"ost{tt}_{c}")
                        nc.vector.tensor_copy(ot[:], pss[c][:])
                        nc.sync.dma_start(
                            out_tiled[tt][:, c * HF : (c + 1) * HF], ot[:]
                        )

    nc.compile()
    return nc


_NC = None


def _get_nc():
    global _NC
    if _NC is None:
        _NC = build()
    return _NC


def _run(inputs, trace=False, **kwargs):
    tokens = np.ascontiguousarray(inputs["tokens"], dtype=np.float32)
    Wq = np.ascontiguousarray(inputs["Wq"], dtype=np.float32)
    Wk = np.ascontiguousarray(inputs["Wk"], dtype=np.float32)
    Wv = np.ascontiguousarray(inputs["Wv"], dtype=np.float32)
    assert tokens.shape == (B, T, H)
    nc = _get_nc()
    in_maps = [
        {"tokens": tokens[i], "Wq": Wq, "Wk": Wk, "Wv": Wv} for i in range(N_CORES)
    ]
    res = run_bass_kernel_spmd(
        nc, in_maps, core_ids=list(range(N_CORES)), trace=trace, **kwargs
    )
    out = np.stack([res.results[i]["out"] for i in range(N_CORES)], axis=0)
    return out.astype(np.float32), res


def kernel(**inputs) -> np.ndarray:
    out, _ = _run(inputs)
    return out



# revision 13
# speedup vs baseline: 1.0735x; 1.0000x over previous
"""Trainium2 Bass kernel for batched attention with softmax over the query axis.

Reference computation (per batch element b):
    Q = tokens @ Wq; K = tokens @ Wk; V = tokens @ Wv
    S = Q @ K.T                [T(t), T(s)]
    A = softmax(S, axis=t)     (normalizes over the *query* axis per key column)
    out = A @ V                [T, H]

Sharding: pure data parallelism - B=8 batch elements, one per NeuronCore.
The softmax couples queries only within a batch element, so no collectives.

Per-core implementation (fp16 matmul operands, fp32 PSUM accumulation):
  - W_qk = Wq @ Wk.T is built on-chip (weight-only work that overlaps the
    token DMA), so scores need one projection G = tokens @ W_qk instead of
    separate Q and K: S = G @ tokens.T.
  - Input DMA: one 1MB push per tensor/stage (wk, wq, tok sg0, wv, tok
    sg1-3) through a 4-slot ring of [128,2048] f32 stage tiles; slot reuse
    orders token stages 1-3 behind the weight consumers. Input streams at
    ~280-300 GB/s, so the 3MB that gates GT[0] (wk+wq+tok0) lands ~19us
    and GT[0] starts ~23.5us after the evac chain; this is the head floor.
  - HAM warm-up: the PE clock gate defaults to K=4/8 (1.2 GHz) and needs
    ~3.4us of real matmul activity to reach 2.4 GHz; transpose-mode does
    NOT count. Dummy ident matmuls (12 up front from ~7.8us when the
    identity exists, plus small batches between DMA-paced transpose
    groups) keep the HAM window fed so Wqk/GT run warm from the start.
  - Weights are transposed on the PE in f32 straight off the DMA stage (PE
    is DMA-paced idle there; the batched DVE evacuation doubles as the f16
    cast). Tokens are DVE-cast to f16 then PE-transposed (tokT) with one
    batched DVE evac per t-tile.
  - GT   [g%128, g//128, t] via lhsT=W_qk rhs=tokT.
  - V    [s%128, s//128, h] via lhsT=tokT rhs=Wv; evacuated by ScalarE.
    Stage 0 runs GT before V (wv arrives later); stages 1-3 run V first.
  - S_st [s%128, t] via lhsT=tokT rhs=GT -> softmax over t is a free-axis
    reduction: max (DVE), exp (ScalarE, accum_out produces row sums).
  - 1/rowsum is folded into V rows, so the unnormalized exp tile E feeds
    the context matmul: ctx[t,h] via lhsT=E rhs=V' accumulated over s. The
    last ctx tile runs as two FD-256 halves (same cadence, shared LDW per
    step) so half the output is in flight before the final matmul ends.
The matmul stream runs gapless at the 216ns/512-free cadence (measured,
= the fp16 1 col/cycle roofline at 2.4GHz) from GT[0] (~23.5us) to the last
context matmul (~164.6us); S+ctx alone are 110us of irreducible fp16 PE
work. Measured budget: ~7.9us fixed NEFF startup, ~15.6us DMA-bound prep,
~141us PE-dense stream (at theory), ~5.2us tail (evac+push+transfer ~2.6 +
~2.6 finalize). Fast-clock exec: ~169.7us.

Things measured NOT to work (do not retry blindly):
  - GpSimd tensor_copy casts: 1859ns per [128,512] chunk, 4.3x slower
    than DVE -> GpSimd is useless for bulk copies.
  - hb-outer GT accumulation (4 live PSUM tiles): blows the 8-bank ring
    during stages (tr pairs + V + GT > 8) -> ~450ns stall per stage.
  - Pair-batched tokT evacs ([128,4,256] copies): DVE 3D-AP copies run
    ~680ns vs 424ns, net slower than 4 per-tile evacs.
  - Dual-engine tail (ScalarE evac + scalar-queue push for half): small
    DMA pushes have ~1.3us fixed transfer latency; split is net slower.
  - Interleaving Wqk[hb] right after its wq-chunk transpose: PE then
    waits out the PSUM->SBUF evac latency once per chunk (~790ns x4).
  - fp8: e4m3 quantization (~3.6% rms/operand) blows the 2e-2 rel-err
    budget on S (logit errors explode through exp) and likely on ctx;
    DoubleRow also halves nothing at FD=512 once LDW doubles (+72%).

NOTE: the chip flips between a 216ns/512-free cadence (2.4 GHz) and a
259ns cadence (~2.0 GHz P0 power state) RUN-TO-RUN - an identical NEFF
measured 169.7us and 201.4us in consecutive runs. This is environmental
(power/thermal, possibly neighbors), not NEFF-content-deterministic as
previously believed. ALWAYS check the cadence in the trace before
attributing a wall-time delta to a code change.
"""

import numpy as np

import concourse.bass as bass
import concourse.bacc as bacc
import concourse.tile as tile
from concourse import mybir
from concourse.bass_utils import run_bass_kernel_spmd
from concourse.masks import make_identity

B, T, H, E = 8, 2048, 512, 512
P = 128
NT = T // P      # 16 tiles along t / s
NH = H // P      # 4 tiles along h
FD = 512         # matmul moving free dim (one fp32 PSUM bank)
NC_T = T // FD   # 4 free-dim chunks along t
NST = T // FD    # 4 token stage groups (4 t-tiles each)

F32 = mybir.dt.float32
F16 = mybir.dt.float16
AX = mybir.AxisListType
AF = mybir.ActivationFunctionType

N_CORES = 8


def build():
    nc = bacc.Bacc()
    tok_d = nc.declare_dram_parameter("tokens", [T, H], F32, isOutput=False)
    wq_d = nc.declare_dram_parameter("Wq", [H, E], F32, isOutput=False)
    wk_d = nc.declare_dram_parameter("Wk", [H, E], F32, isOutput=False)
    wv_d = nc.declare_dram_parameter("Wv", [H, H], F32, isOutput=False)
    out_d = nc.declare_dram_parameter("out", [T, H], F32, isOutput=True)

    # [p, tt, h]: partition = t%128, stage groups of 4 t-tiles -> 1MB DMAs
    tok_staged = tok_d.rearrange("(sg tt p) h -> sg p tt h", p=P, tt=NT // NST)
    out_tiled = out_d.rearrange("(tt p) h -> tt p h", p=P)

    with tile.TileContext(nc) as tc:
        with (
            tc.tile_pool(name="persist", bufs=1) as pp,
            tc.tile_pool(name="stage", bufs=2) as sp,
            tc.tile_pool(name="w16", bufs=4) as wp,
            tc.tile_pool(name="t16", bufs=6) as tp,
            tc.tile_pool(name="ostage", bufs=3) as osp,
            tc.tile_pool(name="stats", bufs=4) as stp,
            tc.tile_pool(name="psum", bufs=8, space=bass.MemorySpace.PSUM) as psp,
        ):
            ident = pp.tile([P, P], F16, tag="ident")
            make_identity(nc, ident[:])
            ident32 = pp.tile([P, P], F32, tag="ident32")
            make_identity(nc, ident32[:])

            # ---- HAM warm-up: the PE clock gate defaults to K=4/8
            # (1.2 GHz) and only reaches 8/8 after ~3.4us of sustained
            # REAL matmul activity (transpose-mode does not count).
            # Without this the whole prep phase runs at half clock
            # (trace: HAM K=8 only at ts~19us). Dummy ident x ident
            # matmuls bridge the DMA-bound head: a small front batch
            # (ident ready ~7.8us -> first wk chunk ~9.0us), plus 4
            # dummies after each wk-chunk transpose group to keep the
            # HAM activity window fed until Wqk's dense matmuls start.
            # Counts are sized to never head-of-line-block real work.
            ps_warm = psp.tile([P, P], F32, tag="mm", name="ps_warm")

            def warmup(n):
                for _ in range(n):
                    nc.tensor.matmul(ps_warm[:], ident[:], ident[:],
                                     start=True, stop=True)

            warmup(12)

            # ---- input DMA pushes (Sync queue) through ONE ring of 8
            # [128,512] f32 chunk tiles. Slot reuse makes each later DMA wait
            # until the earlier chunk in that slot has been consumed, so the
            # weight chunks get near-exclusive HBM bandwidth up front instead
            # of round-robin sharing with 4MB of token traffic.
            def chunk(name):
                return sp.tile([P, FD], F32, tag="st32", bufs=8, name=name)

            wstages = {"wq": [], "wk": []}
            wtiled = {
                "wq": wq_d.rearrange("(hh p) e -> hh p e", p=P),
                "wk": wk_d.rearrange("(hh p) e -> hh p e", p=P),
            }
            for name in ("wk", "wq"):
                for hh in range(NH):
                    ws = chunk(f"wst_{name}{hh}")
                    nc.sync.dma_start(ws[:], wtiled[name][hh])
                    wstages[name].append(ws)

            tok_chunked = tok_d.rearrange("(sg tt p) h -> sg tt p h", p=P,
                                          tt=NT // NST)
            tchunks = [[None] * (NT // NST) for _ in range(NST)]
            for ti in range(NT // NST):
                tc0 = chunk(f"tok0_{ti}")
                nc.sync.dma_start(tc0[:], tok_chunked[0][ti])
                tchunks[0][ti] = tc0

            wv_tiled = wv_d.rearrange("(hh p) e -> hh p e", p=P)
            wvchunks = []
            for hh in range(NH):
                wc = chunk(f"wv{hh}")
                nc.sync.dma_start(wc[:], wv_tiled[hh])
                wvchunks.append(wc)

            for sg in range(1, NST):
                for ti in range(NT // NST):
                    tcx = chunk(f"tok{sg}_{ti}")
                    nc.sync.dma_start(tcx[:], tok_chunked[sg][ti])
                    tchunks[sg][ti] = tcx

            # ---- Wq/Wk: f32 PE transpose straight off the DMA stage (PE
            # is DMA-paced here; the batched DVE evacuation doubles as
            # the f16 cast). Each W_qk[hb] matmul group is emitted right
            # after its wq-chunk transpose so dense real matmuls start as
            # soon as wq chunks land; wk groups get dummy-matmul filler
            # to keep the HAM window busy during DMA waits.
            wT16 = {
                "wq": pp.tile([P, NH, E], F16, tag="wqT", name="wT_wq"),
                "wk": pp.tile([P, NH, E], F16, tag="wkT", name="wT_wk"),
            }
            Wqk = pp.tile([P, NH, H], F16, tag="Wqk")
            for name in ("wk", "wq"):
                for hh in range(NH):
                    # f32 transpose straight off the stage: PE is DMA-paced
                    # idle there, and it keeps the DVE off the W_qk
                    # critical path (one batched evac per chunk is all).
                    ps_tr = psp.tile([P, NH, P], F32, tag="mm",
                                     name=f"tr_{name}{hh}")
                    for eb in range(NH):
                        nc.tensor.transpose(
                            ps_tr[:, eb],
                            wstages[name][hh][:, eb * P : (eb + 1) * P],
                            ident32[:],
                        )
                    nc.vector.tensor_copy(
                        wT16[name][:, :, hh * P : (hh + 1) * P], ps_tr[:]
                    )
                    # fill the DMA wait for the next chunk with real
                    # (HAM-visible) matmul activity; transposes alone do
                    # not count as PE-busy for the HAM clock gate
                    warmup(4 if name == "wk" else 2)

            # ---- W_qk = Wq @ Wk.T : [h%128, hb, h'] fp16 ----
            # Emitted batched after all transposes: interleaving it per
            # wq chunk was measured slower (the PE then waits out the
            # PSUM->SBUF evac latency once per chunk instead of once).
            for hb in range(NH):
                ps = psp.tile([P, FD], F32, tag="mm", name=f"ps_wqk{hb}")
                for eb in range(NH):
                    nc.tensor.matmul(
                        ps[:],
                        wT16["wq"][:, eb, hb * P : (hb + 1) * P],
                        wT16["wk"][:, eb, :],
                        start=(eb == 0),
                        stop=(eb == NH - 1),
                    )
                nc.scalar.copy(Wqk[:, hb, :], ps[:])

            # Token stage 0 casts early in the DVE queue (GpSimd measured
            # 4.3x slower per cast, so they stay on the DVE).
            t16s0 = []
            for ti in range(NT // NST):
                t16 = tp.tile([P, H], F16, tag="t16", name=f"t16_{ti}")
                nc.vector.tensor_copy(t16[:], tchunks[0][ti][:])
                t16s0.append(t16)

            wv16 = pp.tile([P, NH, E], F16, tag="wv16")

            # ---- per stage: transposes -> GT chunk -> V tiles ----
            tokT = pp.tile([P, NH, T], F16, tag="tokT")
            GT = pp.tile([P, NH, T], F16, tag="GT")
            V = pp.tile([P, NT, H], F16, tag="V")
            for sg in range(NST):
                # t16 casts for this stage (stage 0 pre-cast above).
                if sg == 0:
                    t16s = t16s0
                else:
                    t16s = []
                    for ti in range(NT // NST):
                        t16 = tp.tile([P, H], F16, tag="t16",
                                      name=f"t16_{sg * (NT // NST) + ti}")
                        nc.vector.tensor_copy(t16[:], tchunks[sg][ti][:])
                        t16s.append(t16)
                for ti in range(NT // NST):
                    tt = sg * (NT // NST) + ti
                    ps_tr = psp.tile([P, NH, P], F16, tag="mm", name=f"trt{tt}")
                    for ht in range(NH):
                        nc.tensor.transpose(
                            ps_tr[:, ht],
                            t16s[ti][:, ht * P : (ht + 1) * P],
                            ident[:],
                        )
                    nc.vector.tensor_copy(
                        tokT[:, :, tt * P : (tt + 1) * P], ps_tr[:]
                    )
                if sg == 0:
                    # wv casts after stage-0 tokT evacs in the DVE queue
                    # so GT[0] isn't stuck behind them.
                    for hh in range(NH):
                        nc.vector.tensor_copy(wv16[:, hh], wvchunks[hh][:])

                def emit_V(sg):
                    for st in range(sg * NC_T, (sg + 1) * NC_T):
                        ps = psp.tile([P, FD], F32, tag="mm", name=f"ps_v{st}")
                        for ht in range(NH):
                            nc.tensor.matmul(
                                ps[:],
                                tokT[:, ht, st * P : (st + 1) * P],
                                wv16[:, ht, :],
                                start=(ht == 0),
                                stop=(ht == NH - 1),
                            )
                        nc.scalar.copy(V[:, st, :], ps[:])

                def emit_GT(sg):
                    tch = sg
                    for gb in range(NH):
                        ps = psp.tile([P, FD], F32, tag="mm",
                                      name=f"ps_g{gb}_{tch}")
                        for hb in range(NH):
                            nc.tensor.matmul(
                                ps[:],
                                Wqk[:, hb, gb * P : (gb + 1) * P],
                                tokT[:, hb, tch * FD : (tch + 1) * FD],
                                start=(hb == 0),
                                stop=(hb == NH - 1),
                            )
                        nc.scalar.copy(GT[:, gb, tch * FD : (tch + 1) * FD],
                                       ps[:])

                if sg == 0:
                    emit_GT(sg)
                    emit_V(sg)
                else:
                    emit_V(sg)
                    emit_GT(sg)

            # ---- scores S[s,t] + softmax over t (free axis) ----
            Etile = pp.tile([P, NT, T], F16, tag="E")
            for st in range(NT):
                pss = [
                    psp.tile([P, FD], F32, tag="mm", name=f"ps_s{st}_{tch}")
                    for tch in range(NC_T)
                ]
                for tch in range(NC_T):
                    for hb in range(NH):
                        nc.tensor.matmul(
                            pss[tch][:],
                            tokT[:, hb, st * P : (st + 1) * P],
                            GT[:, hb, tch * FD : (tch + 1) * FD],
                            start=(hb == 0),
                            stop=(hb == NH - 1),
                        )
                mx4 = stp.tile([P, NC_T], F32, tag="mx4")
                for tch in range(NC_T):
                    nc.vector.reduce_max(
                        mx4[:, tch : tch + 1], pss[tch][:], axis=AX.X
                    )
                nmx = stp.tile([P, 1], F32, tag="nmx")
                nc.vector.reduce_max(nmx[:], mx4[:], axis=AX.X, negate=True)
                racc = stp.tile([P, NC_T], F32, tag="racc")
                for tch in range(NC_T):
                    nc.scalar.activation(
                        Etile[:, st, tch * FD : (tch + 1) * FD],
                        pss[tch][:],
                        AF.Exp,
                        bias=nmx[:],
                        accum_out=racc[:, tch : tch + 1],
                    )
                rsum = stp.tile([P, 1], F32, tag="rsum")
                nc.vector.reduce_sum(rsum[:], racc[:], axis=AX.X)
                rinv = stp.tile([P, 1], F32, tag="rinv")
                nc.vector.reciprocal(rinv[:], rsum[:])
                # Fold 1/rowsum into V rows (rowsum is per-s, V is s-major).
                nc.vector.tensor_scalar_mul(V[:, st, :], V[:, st, :], rinv[:])

            # ---- context: ctx[t,h] = sum_s E[s,t] * V'[s,h] ----
            for tt in range(NT):
                if tt < NT - 1:
                    ps = psp.tile([P, FD], F32, tag="mm", name=f"ps_c{tt}")
                    for st in range(NT):
                        nc.tensor.matmul(
                            ps[:],
                            Etile[:, st, tt * P : (tt + 1) * P],
                            V[:, st, :],
                            start=(st == 0),
                            stop=(st == NT - 1),
                        )
                    ot = osp.tile([P, H], F32, tag="ostage", name=f"ost{tt}")
                    nc.vector.tensor_copy(ot[:], ps[:])
                    nc.sync.dma_start(out_tiled[tt], ot[:])
                else:
                    # Last tile: two FD-256 halves sharing each st's
                    # stationary operand (one LDW, two matmuls at the same
                    # ~110ns cadence), so half the output is evacuated and
                    # pushed while the other half finishes accumulating.
                    HF = FD // 2
                    pss = [
                        psp.tile([P, HF], F32, tag="mm", name=f"ps_c{tt}_{c}")
                        for c in range(2)
                    ]
                    for st in range(NT):
                        for c in range(2):
                            nc.tensor.matmul(
                                pss[c][:],
                                Etile[:, st, tt * P : (tt + 1) * P],
                                V[:, st, c * HF : (c + 1) * HF],
                                start=(st == 0),
                                stop=(st == NT - 1),
                            )
                    for c in range(2):
                        ot = osp.tile([P, HF], F32, tag="ostrip",
                                      name=f"ost{tt}_{c}")
                        nc.vector.tensor_copy(ot[:], pss[c][:])
                        nc.sync.dma_start(
                            out_tiled[tt][:, c * HF : (c + 1) * HF], ot[:]
                        )

    nc.compile()
    return nc


_NC = None


def _get_nc():
    global _NC
    if _NC is None:
        _NC = build()
    return _NC


def _run(inputs, trace=False, **kwargs):
    tokens = np.ascontiguousarray(inputs["tokens"], dtype=np.float32)
    Wq = np.ascontiguousarray(inputs["Wq"], dtype=np.float32)
    Wk = np.ascontiguousarray(inputs["Wk"], dtype=np.float32)
    Wv = np.ascontiguousarray(inputs["Wv"], dtype=np.float32)
    assert tokens.shape == (B, T, H)
    nc = _get_nc()
    in_maps = [
        {"tokens": tokens[i], "Wq": Wq, "Wk": Wk, "Wv": Wv} for i in range(N_CORES)
    ]
    res = run_bass_kernel_spmd(
        nc, in_maps, core_ids=list(range(N_CORES)), trace=trace, **kwargs
    )
    out = np.stack([res.results[i]["out"] for i in range(N_CORES)], axis=0)
    return out.astype(np.float32), res


def kernel(**inputs) -> np.ndarray:
    out, _ = _run(inputs)
    return out

